# revision 3
# baseline (speedup 1.0000x reference)
"""BiLSTM (nn_BiLSTM) Trainium2 Bass kernel — 8-core data-parallel on batch.

Strategy per core (B_local = 32 of B = 256):
  - Gather the 6400 = T*32 embedding rows via indirect DMA into [128, 300+pad]
    tiles (4 timesteps per tile, t-major).
  - PE-transpose each tile's E-chunks -> xsT [E(part), rows] and matmul with
    W_x chunks to precompute xw^T = (x_t @ W_x)^T for all t, stored in SBUF
    interleaved per step: xwT[:, t, gate, b] with gate order (j, i, f, o).
    The ACT copy out of PSUM folds in the bias (plus forget_bias on f).
  - Sequential forward LSTM over 200 steps in transposed state layout
    (hT, c: [H=128 partitions, B=32 free]):
      z-psum[:, g*32:(g+1)*32] = xwT_t (injected via identity matmul) +
                                 W_h[:, g-block]^T-matmuls against hT.
      ACT: tanh(j), sigmoid([i, f, o]); DVE: c = sf*c + si*tj; ACT tanh(c);
      DVE: hT = so*tc.
  - The "backward" direction contribution to the output is just ONE LSTM step
    on x[:, T-1] from zero state (bw_hs[0] of the reversed scan), so no
    backward scan at all: c_bw = sig(i)*tanh(j), h_bw = sig(o)*tanh(c_bw).
  - scores^T = w_out^T @ [h_fw; h_bw] + b_out, emitted as [6, 32]; host
    transposes/concats.
All precompute (gather/transpose/xw matmuls) is software-pipelined into the
idle engine slots of the sequential recurrence via interleaved emission.
"""

import numpy as np

import concourse.bass as bass
import concourse.mybir as mybir
import concourse.tile as tile
from concourse import bacc
from concourse.bass_utils import run_bass_kernel_spmd

FP = mybir.dt.float32
I32 = mybir.dt.int32

# Problem constants
B, T_FULL, V, E, H, C = 256, 200, 50000, 300, 128, 6
NCORES = 8
BL = B // NCORES            # 32 rows per core
TPB = 128 // BL             # 4 timesteps per gather tile
EC = 3                      # ceil(300/128) E chunks
EPAD = EC * 128             # 384
BLK = 4                     # gather tiles per stage-2 block
PERM = (1, 0, 2, 3)         # reference gate order (i,j,f,o) -> (j,i,f,o)
GJ, GI, GF, GO = 0, 1, 2, 3  # gate slots in permuted order


def build_nc(T=T_FULL):
    RT = (T + TPB - 1) // TPB          # gather tiles (T=200 -> 50)
    assert RT * TPB == T
    NBLK = (RT + BLK - 1) // BLK       # stage-2 blocks (13)
    SPB = TPB * BLK                    # steps per full block (16)

    nc = bacc.Bacc("TRN2", target_bir_lowering=False, debug=False,
                   num_devices=NCORES)

    emb = nc.dram_tensor("emb", [V, E], FP, kind="ExternalInput")
    idx_d = nc.dram_tensor("idx", [128, RT], I32, kind="ExternalInput")
    wx_fw_d = nc.dram_tensor("wx_fw", [128, EC, 4 * H], FP, kind="ExternalInput")
    wh_fw_d = nc.dram_tensor("wh_fw", [128, 4 * H], FP, kind="ExternalInput")
    b_fw_d = nc.dram_tensor("b_fw", [128, 4], FP, kind="ExternalInput")
    wx_bw_d = nc.dram_tensor("wx_bw", [128, EC, 4 * H], FP, kind="ExternalInput")
    b_bw_d = nc.dram_tensor("b_bw", [128, 4], FP, kind="ExternalInput")
    wout_d = nc.dram_tensor("wout", [128, 2, C], FP, kind="ExternalInput")
    bout_d = nc.dram_tensor("bout", [C, 1], FP, kind="ExternalInput")
    ident_d = nc.dram_tensor("ident", [128, 128], FP, kind="ExternalInput")
    out_d = nc.dram_tensor("out", [C, BL], FP, kind="ExternalOutput")

    with tile.TileContext(nc) as tc:
        with (
            tc.tile_pool(name="const", bufs=1) as cpool,
            tc.tile_pool(name="xs", bufs=10) as xs_pool,
            tc.tile_pool(name="xsT", bufs=2) as xsT_pool,
            tc.tile_pool(name="small", bufs=2) as sp,
            tc.tile_pool(name="pT", bufs=2, space="PSUM") as pT_pool,
            tc.tile_pool(name="ps2", bufs=4, space="PSUM") as ps2_pool,
            tc.tile_pool(name="pz", bufs=2, space="PSUM") as pz_pool,
        ):
            # ---- persistent SBUF: constants + xwT ----
            idx_sb = cpool.tile([128, RT], I32, tag="idx")
            wx_sb = cpool.tile([128, EC, 4 * H], FP, tag="wx")
            wh_sb = cpool.tile([128, 4 * H], FP, tag="wh")
            bf_sb = cpool.tile([128, 4], FP, tag="bf")
            wxb_sb = cpool.tile([128, EC, 4 * H], FP, tag="wxb")
            bb_sb = cpool.tile([128, 4], FP, tag="bb")
            wo_sb = cpool.tile([128, 2, C], FP, tag="wo")
            bo_sb = cpool.tile([C, 1], FP, tag="bo")
            id_sb = cpool.tile([128, 128], FP, tag="id")
            xwT = cpool.tile([128, T, 4, BL], FP, tag="xwT")

            nc.sync.dma_start(out=idx_sb[:], in_=idx_d[:])
            nc.sync.dma_start(out=wx_sb[:], in_=wx_fw_d[:])
            nc.sync.dma_start(out=wh_sb[:], in_=wh_fw_d[:])
            nc.sync.dma_start(out=bf_sb[:], in_=b_fw_d[:])
            nc.sync.dma_start(out=wxb_sb[:], in_=wx_bw_d[:])
            nc.sync.dma_start(out=bb_sb[:], in_=b_bw_d[:])
            nc.sync.dma_start(out=wo_sb[:], in_=wout_d[:])
            nc.sync.dma_start(out=bo_sb[:], in_=bout_d[:])
            nc.sync.dma_start(out=id_sb[:], in_=ident_d[:])

            xs_tiles = {}       # q -> xs tile
            xsT_blks = {}       # k -> xsT tile [128, EC, BLK*128]

            def blk_tiles(k):
                return min(BLK, RT - k * BLK)

            def g_ops(q):
                """Gather 128 embedding rows for tile q; zero the E pad."""
                t_ = xs_pool.tile([128, EPAD], FP, tag="xs", name="xs")
                xs_tiles[q] = t_
                nc.gpsimd.indirect_dma_start(
                    out=t_[:, 0:E], out_offset=None,
                    in_=emb[:, :],
                    in_offset=bass.IndirectOffsetOnAxis(
                        ap=idx_sb[:, q:q + 1], axis=0),
                )
                nc.gpsimd.memset(t_[:, E:EPAD], 0.0)

            def t_ops(q):
                """PE-transpose tile q's 3 E-chunks; copy into its block xsT."""
                k = q // BLK
                if k not in xsT_blks:
                    xsT_blks[k] = xsT_pool.tile([128, EC, BLK * 128], FP,
                                                tag="xsT", name="xsT")
                xst = xsT_blks[k]
                xq = xs_tiles.pop(q)
                pt = pT_pool.tile([128, EC, 128], FP, tag="pT", name="pT")
                for e in range(EC):
                    nc.tensor.transpose(pt[:, e, :], xq[:, e * 128:(e + 1) * 128],
                                        id_sb[:])
                qq = q % BLK
                nc.vector.tensor_copy(xst[:, :, qq * 128:(qq + 1) * 128], pt[:])

            def s2_gate(k, g):
                """Stage-2: xw^T for block k, gate g; ACT copy folds bias."""
                ncols = blk_tiles(k) * 128
                nsteps = blk_tiles(k) * TPB
                xst = xsT_blks[k]
                ps = ps2_pool.tile([128, 512], FP, tag="ps2", name="ps2")
                for e in range(EC):
                    nc.tensor.matmul(
                        ps[:, 0:ncols],
                        wx_sb[:, e, g * H:(g + 1) * H],
                        xst[:, e, 0:ncols],
                        start=(e == 0), stop=(e == EC - 1))
                nc.scalar.activation(
                    xwT[:, k * SPB:k * SPB + nsteps, g, :],
                    ps[:, 0:ncols].rearrange("p (s b) -> p s b", b=BL),
                    mybir.ActivationFunctionType.Identity,
                    bias=bf_sb[:, g:g + 1])

            # ---------------- prologue ----------------
            for q in range(min(2 * BLK, RT)):
                g_ops(q)
            for q in range(min(BLK, RT)):
                t_ops(q)
            for g in range(4):
                s2_gate(0, g)
            for q in range(2 * BLK, min(3 * BLK, RT)):
                g_ops(q)
            for q in range(BLK, min(2 * BLK, RT)):
                t_ops(q)

            # ---------------- recurrence ----------------
            Tanh = mybir.ActivationFunctionType.Tanh
            Sig = mybir.ActivationFunctionType.Sigmoid
            Ident = mybir.ActivationFunctionType.Identity
            mult = mybir.AluOpType.mult
            add = mybir.AluOpType.add

            zb = pz_pool.tile([128, 128], FP, tag="z", name="z")
            nc.tensor.matmul(zb[:], id_sb[:], xwT[:, 0, :, :],
                             start=True, stop=True)
            h_prev = c_prev = None
            for t in range(T):
                k, pos = t // SPB, t % SPB
                if t > 0:
                    for g in range(4):
                        nc.tensor.matmul(
                            zb[:, g * BL:(g + 1) * BL],
                            wh_sb[:, g * H:(g + 1) * H],
                            h_prev[:],
                            start=False, stop=(g == 3))
                # inject x-term for t+1 (independent of h; fills PE stalls)
                if t + 1 < T:
                    zb_next = pz_pool.tile([128, 128], FP, tag="z", name="z")
                    nc.tensor.matmul(zb_next[:], id_sb[:], xwT[:, t + 1, :, :],
                                     start=True, stop=False)
                else:
                    zb_next = None
                # background pipeline work, one window per 4 steps
                if pos % TPB == 0:
                    j = pos // TPB
                    q = (k + 3) * BLK + j
                    if q < RT:
                        g_ops(q)
                    q = (k + 2) * BLK + j
                    if q < RT:
                        t_ops(q)
                    if k + 1 < NBLK:
                        s2_gate(k + 1, j)

                # elementwise cell
                tj = sp.tile([128, BL], FP, tag="tj", name="tj")
                nc.scalar.activation(tj[:], zb[:, 0:BL], Tanh)
                sg = sp.tile([128, 3 * BL], FP, tag="sg", name="sg")
                nc.scalar.activation(sg[:], zb[:, BL:4 * BL], Sig)
                c_cur = sp.tile([128, BL], FP, tag="c", name="c")
                if t == 0:
                    nc.vector.tensor_tensor(c_cur[:], sg[:, 0:BL], tj[:], mult)
                else:
                    u = sp.tile([128, BL], FP, tag="u", name="u")
                    nc.vector.tensor_tensor(u[:], sg[:, 0:BL], tj[:], mult)
                    v = sp.tile([128, BL], FP, tag="v", name="v")
                    nc.vector.tensor_tensor(v[:], sg[:, BL:2 * BL], c_prev[:],
                                            mult)
                    nc.vector.tensor_tensor(c_cur[:], u[:], v[:], add)
                tc_ = sp.tile([128, BL], FP, tag="tc", name="tc")
                nc.scalar.activation(tc_[:], c_cur[:], Tanh)
                h_cur = sp.tile([128, BL], FP, tag="h", name="h")
                nc.vector.tensor_tensor(h_cur[:], sg[:, 2 * BL:3 * BL], tc_[:],
                                        mult)
                h_prev, c_prev = h_cur, c_cur
                zb = zb_next

            # ---------------- backward: single step on x[:, T-1] ----------------
            lastk = (RT - 1) // BLK
            xst = xsT_blks[lastk]
            lq = (RT - 1) % BLK
            coff = lq * 128 + (TPB - 1) * BL       # t = T-1 rows
            zbw = pz_pool.tile([128, 128], FP, tag="z", name="z")
            for s, g in ((0, GJ), (1, GI), (2, GO)):
                for e in range(EC):
                    nc.tensor.matmul(
                        zbw[:, s * BL:(s + 1) * BL],
                        wxb_sb[:, e, g * H:(g + 1) * H],
                        xst[:, e, coff:coff + BL],
                        start=(e == 0), stop=(e == EC - 1))
            tjb = sp.tile([128, BL], FP, tag="tj", name="tj")
            nc.scalar.activation(tjb[:], zbw[:, 0:BL], Tanh,
                                 bias=bb_sb[:, GJ:GJ + 1])
            sib = sp.tile([128, BL], FP, tag="u", name="u")
            nc.scalar.activation(sib[:], zbw[:, BL:2 * BL], Sig,
                                 bias=bb_sb[:, GI:GI + 1])
            sob = sp.tile([128, BL], FP, tag="v", name="v")
            nc.scalar.activation(sob[:], zbw[:, 2 * BL:3 * BL], Sig,
                                 bias=bb_sb[:, GO:GO + 1])
            cbw = sp.tile([128, BL], FP, tag="c", name="c")
            nc.vector.tensor_tensor(cbw[:], sib[:], tjb[:], mult)
            tcb = sp.tile([128, BL], FP, tag="tc", name="tc")
            nc.scalar.activation(tcb[:], cbw[:], Tanh)
            hbw = sp.tile([128, BL], FP, tag="sg", name="sg")
            nc.vector.tensor_tensor(hbw[:], sob[:], tcb[:], mult)

            # ---------------- output ----------------
            po = pz_pool.tile([C, BL], FP, tag="z", name="po")
            nc.tensor.matmul(po[:], wo_sb[:, 0, :], h_prev[:],
                             start=True, stop=False)
            nc.tensor.matmul(po[:], wo_sb[:, 1, :], hbw[:],
                             start=False, stop=True)
            out_sb = sp.tile([C, BL], FP, tag="out", name="out")
            nc.scalar.activation(out_sb[:], po[:], Ident, bias=bo_sb[:, 0:1])
            nc.sync.dma_start(out=out_d[:], in_=out_sb[:])

    nc.compile()
    return nc


# ---------------- host-side packing ----------------

def _permute_gates(w):
    """Reorder the trailing 4H axis from (i,j,f,o) to (j,i,f,o)."""
    wg = w.reshape(*w.shape[:-1], 4, H)
    return wg[..., PERM, :].reshape(*w.shape)


def prep_inputs(x, embeds, W_fw, b_fw, W_bw, b_bw, w_out, b_out, T=T_FULL):
    """Returns (shared_dict, per_core_list_of_dicts)."""
    RT = T // TPB
    x = np.asarray(x, np.int32)
    embeds = np.ascontiguousarray(np.asarray(embeds, np.float32))

    def pack_wx(W):
        Wx = _permute_gates(np.asarray(W, np.float32)[:E])
        pad = np.zeros((EPAD, 4 * H), np.float32)
        pad[:E] = Wx
        return np.ascontiguousarray(pad.reshape(EC, 128, 4 * H).transpose(1, 0, 2))

    def pack_b(b_vec, forget_bias):
        bg = np.asarray(b_vec, np.float32).reshape(4, H)[list(PERM)].copy()
        bg[GF] += forget_bias
        return np.ascontiguousarray(bg.T)

    shared = {
        "emb": embeds,
        "wx_fw": pack_wx(W_fw),
        "wh_fw": np.ascontiguousarray(_permute_gates(np.asarray(W_fw, np.float32)[E:])),
        "b_fw": pack_b(b_fw, 1.0),
        "wx_bw": pack_wx(W_bw),
        "b_bw": pack_b(b_bw, 1.0),
        "wout": np.ascontiguousarray(
            np.asarray(w_out, np.float32).reshape(2, H, C).transpose(1, 0, 2)),
        "bout": np.ascontiguousarray(np.asarray(b_out, np.float32).reshape(C, 1)),
        "ident": np.eye(128, dtype=np.float32),
    }
    per_core = []
    for c in range(NCORES):
        xc = x[c * BL:(c + 1) * BL, :T]                      # [32, T]
        idxm = xc.T.reshape(RT, TPB, BL).reshape(RT, 128)    # [RT, 128]
        per_core.append({"idx": np.ascontiguousarray(idxm.T), **shared})
    return per_core


_NC_CACHE = {}


def _get_nc(T=T_FULL):
    if T not in _NC_CACHE:
        _NC_CACHE[T] = build_nc(T)
    return _NC_CACHE[T]


def kernel(x, embeds, W_fw, b_fw, W_bw, b_bw, w_out, b_out):
    nc = _get_nc()
    in_maps = prep_inputs(x, embeds, W_fw, b_fw, W_bw, b_bw, w_out, b_out)
    res = run_bass_kernel_spmd(nc, in_maps, core_ids=list(range(NCORES)))
    out = np.empty((B, C), np.float32)
    for c in range(NCORES):
        out[c * BL:(c + 1) * BL] = res.results[c]["out"].T
    return out
